# revision 28
# baseline (speedup 1.0000x reference)
"""Distributed Trainium2 (Bass/Tile) kernel for the contrastive loss.

Strategy (8 NeuronCores, SPMD, row-sharded similarity matrix):
  Core c owns 512 of the 4096 rows of sim = reps @ reps^T (per l).
  The host rolls the column order by c*512 for each core so a single
  NEFF serves all cores: the self-match column for local row r is
  always column r, and the positive-pair column is always column
  r + 2048.  Each core:
    - loads all 4096 embedding rows as bf16 (natural [row, d] layout),
    - L2-normalizes rows (fused square+row-sum on DVE in bf16 4x mode;
      inv-norm via exp(-0.5*ln(ssq)) on ACT),
    - transposes normalized bf16 rows to [d, row] via PE transpose mode,
    - computes its 512x4096 row-block of sim on PE (bf16, K=D=128),
    - exp(sim/T) in-place in PSUM + fused row-sum on ACT (1536-wide
      chunks amortize the per-instruction accumulator-read cost),
    - computes the positive-pair similarities directly from the
      normalized rows (z_r . z_{r+2048}) on DVE,
    - subtracts exp(1/T) for the (unit) self-similarity term,
    - combines into per-row weighted loss terms, DMAs out [128, 16].
  Host sums the 8 partial tensors -> scalar loss (the all-reduce).
  Normalize-muls are split DVE/GpSimd; input loads prefetch from the
  idle SP DMA queue; the tiny inv-norm activations run at high
  scheduler priority so the next l's front end is never gated.
"""

import numpy as np

TEMP = 0.2
L, B, K, D = 4, 64, 32, 128
N = B * K          # 2048
M = 2 * N          # 4096 rows of sim per l
NCORES = 8
R = M // NCORES    # 512 local rows per core
SEG = M // 128     # 32 row-tiles of 128 per l
INV_T = 1.0 / TEMP

_built = None


def _build():
    global _built
    if _built is not None:
        return _built
    from contextlib import ExitStack

    import concourse.tile as tile
    from concourse import bacc
    import concourse.mybir as mybir
    from concourse.masks import make_identity

    f32 = mybir.dt.float32
    bf16 = mybir.dt.bfloat16
    AF = mybir.ActivationFunctionType
    OP = mybir.AluOpType
    AX = mybir.AxisListType

    # Pin every ACT op to the natural_log_exp_and_others table set (it covers
    # Copy/Exp/Identity/Ln/Square — everything we use), so bacc emits exactly
    # one LoadActFuncSet instead of thrashing ~2.7us loads between sets.
    from concourse import hw_specs as _hw
    _tabs = dict(_hw.get_activation_tables("gen3"))
    _pinned = {
        name: (fns if name == "natural_log_exp_and_others" else frozenset())
        for name, fns in _tabs.items()
    }
    _hw.get_activation_tables.cache_clear()
    _orig = _hw.get_activation_tables.__wrapped__

    def _patched(arch):
        if arch == "gen3":
            return _pinned
        return _orig(arch)

    _hw.get_activation_tables = _patched
    import concourse.bacc as _baccmod
    if hasattr(_baccmod, "get_activation_tables"):
        _baccmod.get_activation_tables = _patched

    nc = bacc.Bacc(None, target_bir_lowering=False)
    emb = nc.dram_tensor("emb_nat", [128, L, SEG, D], bf16, kind="ExternalInput")
    jvl = nc.dram_tensor("jv_local", [R], f32, kind="ExternalInput")
    out = nc.dram_tensor("out_wlp", [128, 4 * L], f32, kind="ExternalOutput")

    with ExitStack() as ctx:
        tc = ctx.enter_context(tile.TileContext(nc))
        singles = ctx.enter_context(tc.tile_pool(name="singles", bufs=1))
        natp = ctx.enter_context(tc.tile_pool(name="nat", bufs=4))
        xtp = ctx.enter_context(tc.tile_pool(name="xt", bufs=16))
        junkp = ctx.enter_context(tc.tile_pool(name="junk", bufs=4))
        statp = ctx.enter_context(tc.tile_pool(name="stat", bufs=12))
        tpp = ctx.enter_context(tc.tile_pool(name="tp", bufs=2, space="PSUM"))
        simp = ctx.enter_context(tc.tile_pool(name="sim", bufs=2, space="PSUM"))

        ident = singles.tile([128, 128], f32)
        make_identity(nc, ident[:])
        identb = singles.tile([128, 128], bf16)
        nc.vector.tensor_copy(identb[:], ident[:])

        w = singles.tile([128, 4], f32)
        nc.sync.dma_start(out=w[:], in_=jvl.rearrange("(rb p) -> p rb", p=128))

        dsum = singles.tile([128, 4 * L, 3], f32)  # per (l,rb): 3 chunk sums
        posb = singles.tile([128, 4 * L], f32)
        denom = singles.tile([128, 4 * L], f32)
        logd = singles.tile([128, 4 * L], f32)
        lp = singles.tile([128, 4 * L], f32)
        wlp = singles.tile([128, 4 * L], f32)

        for l in range(L):
            # quartered DMAs (from the otherwise idle SP queue, so loads
            # for later l prefetch while earlier l computes) — squared-norm
            # accumulation starts after the first 2KB/partition lands
            nat = natp.tile([128, SEG, D], bf16)
            for q in range(4):
                nc.sync.dma_start(
                    out=nat[:, q * 8 : (q + 1) * 8, :],
                    in_=emb[:, l, q * 8 : (q + 1) * 8, :])

            ssq = statp.tile([128, SEG], f32)
            lnssq = statp.tile([128, SEG], f32)
            invn = statp.tile([128, SEG], f32)
            # squared norms (DVE; GpSimd codegen rejects stt-with-accum)
            for s in range(SEG):
                junk = junkp.tile([128, D], bf16)
                nc.vector.scalar_tensor_tensor(
                    out=junk[:], in0=nat[:, s, :], scalar=1.0,
                    in1=nat[:, s, :],
                    op0=OP.mult, op1=OP.mult, accum_out=ssq[:, s : s + 1])
            # inv_norm = exp(-0.5*ln(ssq)); Ln+Exp share one ACT table set.
            # High priority: the whole next-l front chain hangs off these
            # tiny ACT ops, so they must preempt the exp stream.  Chunked
            # in halves so the first normalize-muls start sooner.
            for h in range(2):
                sl = slice(h * 16, (h + 1) * 16)
                with tc.high_priority():
                    nc.scalar.activation(
                        out=lnssq[:, sl], in_=ssq[:, sl], func=AF.Ln)
                    nc.scalar.activation(
                        out=invn[:, sl], in_=lnssq[:, sl], func=AF.Exp,
                        scale=-0.5)
                for s in range(h * 16, (h + 1) * 16):
                    eng = nc.vector if s % 2 == 0 else nc.gpsimd
                    eng.tensor_scalar_mul(
                        nat[:, s, :], nat[:, s, :], invn[:, s : s + 1])

            # positive-pair similarities for this core's own rows
            # (chunks s=0..3 vs s+16), straight from the normalized rows
            for s in range(4):
                junk = junkp.tile([128, D], bf16)
                nc.vector.scalar_tensor_tensor(
                    out=junk[:], in0=nat[:, s, :], scalar=1.0,
                    in1=nat[:, s + 16, :],
                    op0=OP.mult, op1=OP.mult,
                    accum_out=posb[:, l * 4 + s : l * 4 + s + 1])

            # transpose normalized rows into [d, row] chunks of 512 columns
            # (PE transpose mode, bf16: 1 cyc/row)
            xtc = []
            for g in range(8):
                ps = tpp.tile([128, 512], bf16)
                for kk in range(4):
                    s = g * 4 + kk
                    nc.tensor.transpose(
                        ps[:, kk * 128 : (kk + 1) * 128], nat[:, s, :],
                        identb[:])
                xc = xtp.tile([128, 512], bf16)
                nc.vector.tensor_copy(xc[:], ps[:])
                xtc.append(xc)

            # the 512x4096 sim row-block for this l, exp'd in chunks of
            # (1536, 1536, 1024) to amortize ACT init + accum-read costs
            for rb in range(4):
                lr = l * 4 + rb
                lhsT = xtc[0][:, rb * 128 : (rb + 1) * 128]
                for ci, (f0, nch) in enumerate([(0, 3), (3, 3), (6, 2)]):
                    sim = simp.tile([128, 1536], f32)
                    for u in range(nch):
                        nc.tensor.matmul(
                            sim[:, u * 512 : (u + 1) * 512], lhsT,
                            xtc[f0 + u][:],
                            start=True, stop=True)
                    # exp output written back in place (PSUM->PSUM has a
                    # smaller access-latency charge than PSUM->SBUF and the
                    # elementwise values are never read again)
                    nc.scalar.activation(
                        out=sim[:, 0 : 512 * nch], in_=sim[:, 0 : 512 * nch],
                        func=AF.Exp, scale=INV_T,
                        accum_out=dsum[:, lr, ci : ci + 1])

        # tail: per-row loss terms.  The self-similarity of an L2-normalized
        # row is 1 (up to bf16 rounding), so the self term is exp(1/T).
        nc.vector.reduce_sum(out=denom[:], in_=dsum[:], axis=AX.X)
        nc.vector.tensor_scalar_add(denom[:], denom[:], -float(np.exp(INV_T)))
        nc.scalar.activation(out=logd[:], in_=denom[:], func=AF.Ln)
        nc.vector.tensor_scalar_mul(lp[:], posb[:], -INV_T)
        nc.vector.tensor_add(lp[:], lp[:], logd[:])
        for l in range(L):
            nc.vector.tensor_mul(
                wlp[:, l * 4 : (l + 1) * 4], lp[:, l * 4 : (l + 1) * 4], w[:])
        nc.sync.dma_start(out=out[:, :], in_=wlp[:])

    nc.finalize()
    _built = nc
    return nc


def _in_maps(emb_i, emb_j, joint_valid):
    import ml_dtypes

    emb_i = np.asarray(emb_i, dtype=np.float32)
    emb_j = np.asarray(emb_j, dtype=np.float32)
    jv = np.asarray(joint_valid, dtype=np.float32).reshape(-1)
    reps = np.concatenate(
        [emb_i.reshape(L, N, D), emb_j.reshape(L, N, D)], axis=1)  # [L, M, D]
    reps16 = reps.astype(ml_dtypes.bfloat16)
    maps = []
    for c in range(NCORES):
        idx = (np.arange(M) + c * R) % M
        cols = reps16[:, idx, :]  # rolled so local rows sit at columns 0..R-1
        nat = np.ascontiguousarray(
            cols.reshape(L, SEG, 128, D).transpose(2, 0, 1, 3))
        jvl = np.ascontiguousarray(jv[(np.arange(R) + c * R) % N])
        maps.append({"emb_nat": nat, "jv_local": jvl})
    return maps, jv


def _combine(results, jv):
    tot = 0.0
    for r in results:
        tot += float(r["out_wlp"].astype(np.float64).sum())
    return np.float32(tot / (2.0 * float(jv.sum())))


def kernel(emb_i, emb_j, joint_valid):
    from concourse.bass_utils import run_bass_kernel_spmd

    nc = _build()
    maps, jv = _in_maps(emb_i, emb_j, joint_valid)
    res = run_bass_kernel_spmd(nc, maps, core_ids=list(range(NCORES)))
    return _combine(res.results, jv)


def run_traced(inputs, trace_cores=None):
    """test.py helper: same run but with NTFF tracing enabled."""
    from concourse.bass_utils import run_bass_kernel_spmd

    nc = _build()
    maps, jv = _in_maps(**inputs)
    res = run_bass_kernel_spmd(
        nc, maps, core_ids=list(range(NCORES)), trace=True,
        trace_cores=trace_cores if trace_cores is not None else list(range(NCORES)))
    res.loss = _combine(res.results, jv)
    return res


# revision 33
# speedup vs baseline: 1.0141x; 1.0141x over previous
"""Distributed Trainium2 (Bass/Tile) kernel for the contrastive loss.

Strategy (8 NeuronCores, SPMD, row-sharded similarity matrix):
  Core c owns 512 of the 4096 rows of sim = reps @ reps^T (per l).
  The host rolls the column order by c*512 for each core so a single
  NEFF serves all cores: the self-match column for local row r is
  always column r, and the positive-pair column is always column
  r + 2048.  Each core:
    - loads all 4096 embedding rows as bf16 (natural [row, d] layout),
    - L2-normalizes rows (fused square+row-sum on DVE in bf16 4x mode;
      inv-norm via exp(-0.5*ln(ssq)) on ACT),
    - transposes normalized bf16 rows to [d, row] via PE transpose mode,
    - computes its 512x4096 row-block of sim on PE (bf16, K=D=128),
    - exp(sim/T) in-place in PSUM + fused row-sum on ACT (1536-wide
      chunks amortize the per-instruction accumulator-read cost),
    - computes the positive-pair similarities directly from the
      normalized rows (z_r . z_{r+2048}) on DVE,
    - subtracts exp(1/T) for the (unit) self-similarity term,
    - combines into per-row weighted loss terms, DMAs out [128, 16].
  Host sums the 8 partial tensors -> scalar loss (the all-reduce).
  Normalize-muls are split DVE/GpSimd; input loads prefetch from the
  idle SP DMA queue; the tiny inv-norm activations run at high
  scheduler priority so the next l's front end is never gated.
"""

import numpy as np

TEMP = 0.2
L, B, K, D = 4, 64, 32, 128
N = B * K          # 2048
M = 2 * N          # 4096 rows of sim per l
NCORES = 8
R = M // NCORES    # 512 local rows per core
SEG = M // 128     # 32 row-tiles of 128 per l
INV_T = 1.0 / TEMP

_built = None


def _build():
    global _built
    if _built is not None:
        return _built
    from contextlib import ExitStack

    import concourse.tile as tile
    from concourse import bacc
    import concourse.mybir as mybir
    from concourse.masks import make_identity

    f32 = mybir.dt.float32
    bf16 = mybir.dt.bfloat16
    AF = mybir.ActivationFunctionType
    OP = mybir.AluOpType
    AX = mybir.AxisListType

    # Pin every ACT op to the natural_log_exp_and_others table set (it covers
    # Copy/Exp/Identity/Ln/Square — everything we use), so bacc emits exactly
    # one LoadActFuncSet instead of thrashing ~2.7us loads between sets.
    from concourse import hw_specs as _hw
    _tabs = dict(_hw.get_activation_tables("gen3"))
    _pinned = {
        name: (fns if name == "natural_log_exp_and_others" else frozenset())
        for name, fns in _tabs.items()
    }
    _hw.get_activation_tables.cache_clear()
    _orig = _hw.get_activation_tables.__wrapped__

    def _patched(arch):
        if arch == "gen3":
            return _pinned
        return _orig(arch)

    _hw.get_activation_tables = _patched
    import concourse.bacc as _baccmod
    if hasattr(_baccmod, "get_activation_tables"):
        _baccmod.get_activation_tables = _patched

    nc = bacc.Bacc(None, target_bir_lowering=False)
    emb = nc.dram_tensor("emb_nat", [128, L, SEG, D], bf16, kind="ExternalInput")
    jvl = nc.dram_tensor("jv_local", [R], f32, kind="ExternalInput")
    out = nc.dram_tensor("out_wlp", [128, 4 * L], f32, kind="ExternalOutput")

    with ExitStack() as ctx:
        tc = ctx.enter_context(tile.TileContext(nc))
        singles = ctx.enter_context(tc.tile_pool(name="singles", bufs=1))
        natp = ctx.enter_context(tc.tile_pool(name="nat", bufs=4))
        xtp = ctx.enter_context(tc.tile_pool(name="xt", bufs=16))
        junkp = ctx.enter_context(tc.tile_pool(name="junk", bufs=4))
        statp = ctx.enter_context(tc.tile_pool(name="stat", bufs=12))
        tpp = ctx.enter_context(tc.tile_pool(name="tp", bufs=2, space="PSUM"))
        simp = ctx.enter_context(tc.tile_pool(name="sim", bufs=2, space="PSUM"))

        ident = singles.tile([128, 128], f32)
        make_identity(nc, ident[:])
        identb = singles.tile([128, 128], bf16)
        nc.vector.tensor_copy(identb[:], ident[:])

        w = singles.tile([128, 4], f32)
        # issue from the gpsimd queue so it doesn't delay the first input
        # load on the SP queue
        nc.gpsimd.dma_start(out=w[:], in_=jvl.rearrange("(rb p) -> p rb", p=128))

        dsum = singles.tile([128, 4 * L, 3], f32)  # per (l,rb): 3 chunk sums
        posb = singles.tile([128, 4 * L], f32)
        denom = singles.tile([128, 4 * L], f32)
        logd = singles.tile([128, 4 * L], f32)
        lp = singles.tile([128, 4 * L], f32)
        wlp = singles.tile([128, 4 * L], f32)

        for l in range(L):
            # chunked DMAs (from the otherwise idle SP queue, so loads
            # for later l prefetch while earlier l computes) — squared-norm
            # accumulation starts after the first chunk lands.  l=0 leads
            # with a small chunk to shorten the pipeline-fill chain.
            nat = natp.tile([128, SEG, D], bf16)
            s0 = 0
            for nseg in ([4, 8, 8, 12] if l == 0 else [8, 8, 8, 8]):
                nc.sync.dma_start(
                    out=nat[:, s0 : s0 + nseg, :],
                    in_=emb[:, l, s0 : s0 + nseg, :])
                s0 += nseg

            ssq = statp.tile([128, SEG], f32)
            lnssq = statp.tile([128, SEG], f32)
            invn = statp.tile([128, SEG], f32)
            # squared norms (DVE; GpSimd codegen rejects stt-with-accum)
            for s in range(SEG):
                junk = junkp.tile([128, D], bf16)
                nc.vector.scalar_tensor_tensor(
                    out=junk[:], in0=nat[:, s, :], scalar=1.0,
                    in1=nat[:, s, :],
                    op0=OP.mult, op1=OP.mult, accum_out=ssq[:, s : s + 1])
            # inv_norm = exp(-0.5*ln(ssq)); Ln+Exp share one ACT table set.
            # High priority: the whole next-l front chain hangs off these
            # tiny ACT ops, so they must preempt the exp stream.  Chunked
            # so the first normalize-muls start sooner (finest for l=0,
            # where the chain is the pipeline-fill critical path).
            nchunk = 4 if l == 0 else 2
            cw = SEG // nchunk
            for h in range(nchunk):
                sl = slice(h * cw, (h + 1) * cw)
                with tc.high_priority():
                    nc.scalar.activation(
                        out=lnssq[:, sl], in_=ssq[:, sl], func=AF.Ln)
                    nc.scalar.activation(
                        out=invn[:, sl], in_=lnssq[:, sl], func=AF.Exp,
                        scale=-0.5)
                for s in range(h * cw, (h + 1) * cw):
                    eng = nc.vector if s % 2 == 0 else nc.gpsimd
                    eng.tensor_scalar_mul(
                        nat[:, s, :], nat[:, s, :], invn[:, s : s + 1])

            # positive-pair similarities for this core's own rows
            # (chunks s=0..3 vs s+16), straight from the normalized rows
            for s in range(4):
                junk = junkp.tile([128, D], bf16)
                nc.vector.scalar_tensor_tensor(
                    out=junk[:], in0=nat[:, s, :], scalar=1.0,
                    in1=nat[:, s + 16, :],
                    op0=OP.mult, op1=OP.mult,
                    accum_out=posb[:, l * 4 + s : l * 4 + s + 1])

            # transpose normalized rows into [d, row] chunks of 512 columns
            # (PE transpose mode, bf16: 1 cyc/row).  For l=0 the first two
            # chunks feed the very first exp, so they jump the queue ahead
            # of the remaining squared-norm/normalize work.
            xtc = []
            for g in range(8):
                from contextlib import nullcontext
                prio = (tc.high_priority() if (l == 0 and g < 2)
                        else nullcontext())
                with prio:
                    ps = tpp.tile([128, 512], bf16)
                    for kk in range(4):
                        s = g * 4 + kk
                        nc.tensor.transpose(
                            ps[:, kk * 128 : (kk + 1) * 128], nat[:, s, :],
                            identb[:])
                    xc = xtp.tile([128, 512], bf16)
                    nc.vector.tensor_copy(xc[:], ps[:])
                    xtc.append(xc)

            # the 512x4096 sim row-block for this l, exp'd in chunks of
            # (1536, 1536, 1024) to amortize ACT init + accum-read costs
            for rb in range(4):
                lr = l * 4 + rb
                lhsT = xtc[0][:, rb * 128 : (rb + 1) * 128]
                for ci, (f0, nch) in enumerate([(0, 2), (2, 3), (5, 3)]):
                    sim = simp.tile([128, 1536], f32)
                    for u in range(nch):
                        nc.tensor.matmul(
                            sim[:, u * 512 : (u + 1) * 512], lhsT,
                            xtc[f0 + u][:],
                            start=True, stop=True)
                    # exp output written back in place (PSUM->PSUM has a
                    # smaller access-latency charge than PSUM->SBUF and the
                    # elementwise values are never read again)
                    nc.scalar.activation(
                        out=sim[:, 0 : 512 * nch], in_=sim[:, 0 : 512 * nch],
                        func=AF.Exp, scale=INV_T,
                        accum_out=dsum[:, lr, ci : ci + 1])

        # tail: per-row loss terms.  The self-similarity of an L2-normalized
        # row is 1 (up to bf16 rounding), so the self term is exp(1/T).
        nc.vector.reduce_sum(out=denom[:], in_=dsum[:], axis=AX.X)
        nc.vector.tensor_scalar_add(denom[:], denom[:], -float(np.exp(INV_T)))
        nc.scalar.activation(out=logd[:], in_=denom[:], func=AF.Ln)
        nc.vector.tensor_scalar_mul(lp[:], posb[:], -INV_T)
        nc.vector.tensor_add(lp[:], lp[:], logd[:])
        for l in range(L):
            nc.vector.tensor_mul(
                wlp[:, l * 4 : (l + 1) * 4], lp[:, l * 4 : (l + 1) * 4], w[:])
        nc.sync.dma_start(out=out[:, :], in_=wlp[:])

    nc.finalize()
    _built = nc
    return nc


def _in_maps(emb_i, emb_j, joint_valid):
    import ml_dtypes

    emb_i = np.asarray(emb_i, dtype=np.float32)
    emb_j = np.asarray(emb_j, dtype=np.float32)
    jv = np.asarray(joint_valid, dtype=np.float32).reshape(-1)
    reps = np.concatenate(
        [emb_i.reshape(L, N, D), emb_j.reshape(L, N, D)], axis=1)  # [L, M, D]
    reps16 = reps.astype(ml_dtypes.bfloat16)
    maps = []
    for c in range(NCORES):
        idx = (np.arange(M) + c * R) % M
        cols = reps16[:, idx, :]  # rolled so local rows sit at columns 0..R-1
        nat = np.ascontiguousarray(
            cols.reshape(L, SEG, 128, D).transpose(2, 0, 1, 3))
        jvl = np.ascontiguousarray(jv[(np.arange(R) + c * R) % N])
        maps.append({"emb_nat": nat, "jv_local": jvl})
    return maps, jv


def _combine(results, jv):
    tot = 0.0
    for r in results:
        tot += float(r["out_wlp"].astype(np.float64).sum())
    return np.float32(tot / (2.0 * float(jv.sum())))


def kernel(emb_i, emb_j, joint_valid):
    from concourse.bass_utils import run_bass_kernel_spmd

    nc = _build()
    maps, jv = _in_maps(emb_i, emb_j, joint_valid)
    res = run_bass_kernel_spmd(nc, maps, core_ids=list(range(NCORES)))
    return _combine(res.results, jv)


def run_traced(inputs, trace_cores=None):
    """test.py helper: same run but with NTFF tracing enabled."""
    from concourse.bass_utils import run_bass_kernel_spmd

    nc = _build()
    maps, jv = _in_maps(**inputs)
    res = run_bass_kernel_spmd(
        nc, maps, core_ids=list(range(NCORES)), trace=True,
        trace_cores=trace_cores if trace_cores is not None else list(range(NCORES)))
    res.loss = _combine(res.results, jv)
    return res
